# revision 5
# baseline (speedup 1.0000x reference)
"""Trainium2 Bass kernel for nn_Attention_3513283248742.

Bilinear attention: scores = h @ W @ b^T, attn = softmax(scores, -1),
ctx = attn @ b.  Shapes: b [32,1024,1024], h [32,256,1024], W_b [1,1024,1024].

Sharding: data-parallel over batch B=32 across 8 NeuronCores (4 batches per
core); W replicated.  No collectives.

v4 over v2 (110.7us baseline):
  * attnT transposes move from the PE to the DMA xbar (one
    dma_start(transpose=True) per batch on the ACT HWDGE queue), cutting
    ~6us of PE time; the PE runs only matmuls (plus a tiny HAM warmup).
  * All 16 hWT groups (4 batches) run FIRST: their DMA deps are just
    W (2MB) + hT (2MB), so the PE has ~27us of work queued by t=12us while
    the 16MB of bT/b streams far ahead of the scores/ctx consumers.
  * Every input is host-packed so each DMA writes 4-16KB contiguous rows
    per partition (v2's 512B-row descriptors ran the ramp at half rate).
  * No SBUF-reader gates and bufs=4 on the bT/b pools: the input queue is
    pure FIFO with zero instruction-level waits, so it never head-of-line
    blocks (v3's pool-reuse waits starved the late batches).
  * PE stream after the hWT phase: s0 s1 ctx0 s2 ctx1 s3 ctx2 ctx3 -- each
    batch's softmax+xbar latency hides behind the next batch's scores.

Per-core pipeline (per batch i):
  hWT  = W^T @ hT_i           lhsT = W chunks, rhs = hT (from host)
  S    = hWT^T @ bT_i         scores [q,k]
  softmax over k: DVE row max, ACT exp (+rowsum via accum), DVE recip
  attnT = DMA xbar transpose of E (fp16, SBUF->SBUF)
  ctx  = attnT^T @ b_i        rhs = b natural layout (from host)
  out  = ctx * invS           ACT epilogue, fp16, DMA'd from the ACT queue
"""

import numpy as np

import concourse.bass as bass
import concourse.mybir as mybir
import concourse.tile as tile
from concourse.bass_utils import run_bass_kernel_spmd
from concourse.vector_clock import ScopedClock

F32 = mybir.dt.float32
F16 = mybir.dt.float16

N_CORES = 8
B, TB, TH, D = 32, 1024, 256, 1024
BPC = B // N_CORES  # batches per core = 4
P = 128
NDC = D // P   # 8 chunks of the D axis
NKC = TB // P  # 8 chunks of the k axis
NQ = TH // P   # 2 chunks of the q axis

_PATCHED = False
CLEAR_SEMS_ON_EXIT = False


def _patch_tile_drain(max_waits_per_inst: int = 1):
    """This walrus build rejects >1 sem wait on the SP Drain instruction that
    TileContext emits on exit; split the waits across preceding sync nops."""
    global _PATCHED
    if _PATCHED:
        return
    _PATCHED = True

    def _drain_and_barrier(self, tick_clock, wait_clock):
        nc = self.nc
        drain_inst = nc.sync.drain()
        wait_clock.add_sem_waits(
            drain_inst.ins, ScopedClock({None: tick_clock.global_clock})
        )
        si = drain_inst.ins.sync_info
        if si is not None and si.on_wait and len(si.on_wait) > max_waits_per_inst:
            waits = list(si.on_wait)
            bb = nc.cur_bb.bb
            assert bb.instructions[-1] is drain_inst.ins
            bb.instructions.pop()
            si.on_wait = waits[:max_waits_per_inst]
            rest = waits[max_waits_per_inst:]
            for i in range(0, len(rest), max_waits_per_inst):
                nop = nc.sync.nop(nofuse=True)
                chunk = rest[i : i + max_waits_per_inst]
                if nop.ins.sync_info is None:
                    nop.ins.sync_info = mybir.SyncInfo(on_wait=chunk, on_update=[])
                else:
                    nop.ins.sync_info.on_wait.extend(chunk)
            bb.instructions.append(drain_inst.ins)
        nc.all_engine_barrier()
        assert self.sems is not None
        popped = nc._tile_sem_poison_stack.pop()
        assert popped is self._sem_poison
        if CLEAR_SEMS_ON_EXIT:
            nc.clear_and_free_semaphores(list(self.sems.allocated().values()))
            nc.all_engine_barrier()
        else:
            nc._state.prepend_free_semaphores(
                [
                    s.num if hasattr(s, "num") else s
                    for s in self.sems.allocated().values()
                ]
            )

    tile.TileContext._drain_and_barrier = _drain_and_barrier


def _split_excess_waits(nc, max_waits: int = 1):
    """Walrus rejects instructions carrying more than `max_waits` sem waits.
    Hoist excess waits onto same-engine nops inserted just before."""
    for f in nc.m.functions:
        for bb in f.blocks:
            out = []
            for ins in list(bb.instructions):
                si = ins.sync_info
                if si is not None and si.on_wait and len(si.on_wait) > max_waits:
                    waits = list(si.on_wait)
                    si.on_wait = waits[:max_waits]
                    rest = waits[max_waits:]
                    for i in range(0, len(rest), max_waits):
                        nop = nc.engines[ins.engine].nop(nofuse=True)
                        cur_bb = nc.cur_bb.bb
                        assert cur_bb.instructions[-1] is nop.ins
                        cur_bb.instructions.pop()
                        nop.ins.sync_info = mybir.SyncInfo(
                            on_wait=rest[i : i + max_waits], on_update=[]
                        )
                        out.append(nop.ins)
                out.append(ins)
            bb.instructions[:] = out


def build_nc():
    _patch_tile_drain()
    nc = bass.Bass(trn_type="TRN2", target_bir_lowering=False, debug=False)
    # all inputs host-packed so each DMA is [P, ...] with >=4KB contiguous
    # per partition on BOTH sides (fast descriptors)
    b_ext = nc.declare_dram_parameter("b", [BPC, P, NKC, D], F16, isOutput=False)
    bt_ext = nc.declare_dram_parameter("bT", [BPC, P, NDC, TB], F16, isOutput=False)
    ht_ext = nc.declare_dram_parameter("hT", [BPC, P, NDC, TH], F16, isOutput=False)
    w_ext = nc.declare_dram_parameter("w", [4, P, NDC, 256], F16, isOutput=False)
    ident_ext = nc.declare_dram_parameter("ident", [P, P], F16, isOutput=False)
    out_ext = nc.declare_dram_parameter("out", [BPC, TH, D], F16, isOutput=True)

    with tile.TileContext(nc) as tc:
        with (
            tc.tile_pool(name="consts", bufs=1) as consts,
            tc.tile_pool(name="bpool", bufs=4) as bpool,
            tc.tile_pool(name="btpool", bufs=4) as btpool,
            tc.tile_pool(name="hwtpool", bufs=4) as hwtpool,
            tc.tile_pool(name="epool", bufs=2) as epool,
            tc.tile_pool(name="atpool", bufs=2) as atpool,
            tc.tile_pool(name="ctxpool", bufs=2) as ctxpool,
            tc.tile_pool(name="stats", bufs=2) as stats,
            tc.tile_pool(name="psbig", bufs=2, space="PSUM") as psbig,
            tc.tile_pool(name="pshw", bufs=2, space="PSUM") as pshw,
            tc.tile_pool(name="pswarm", bufs=2, space="PSUM") as pswarm,
        ):
            # ident on the (otherwise idle at t=0) scalar DMA queue so the
            # warmup transposes can start as soon as the preamble ends.
            ident_t = consts.tile([P, P], F16)
            nc.scalar.dma_start(ident_t[:], ident_ext.ap())
            ident16 = ident_t[:]

            # W in SBUF chunk-major: [din(part), chunk, j, dout256]
            w16 = consts.tile([P, 4, NDC, 256], F16)
            # all four hT batches share one tile, one DMA per batch
            ht_t = consts.tile([P, BPC, NDC, TH], F16)

            # --- input stream: pure FIFO on the sync queue, no waits ---
            bT = [None] * BPC
            bN = [None] * BPC
            for c in range(4):
                nc.sync.dma_start(w16[:, c], w_ext[c])
            for i in range(BPC):
                nc.sync.dma_start(ht_t[:, i], ht_ext[i])
            for i in range(BPC):
                bT[i] = btpool.tile([P, NDC, TB], F16, name=f"bT{i}", tag="bT")
                nc.sync.dma_start(bT[i][:], bt_ext[i])
                bN[i] = bpool.tile([P, NKC, D], F16, name=f"b{i}", tag="b")
                nc.sync.dma_start(bN[i][:], b_ext[i])

            # --- PE warmup: trip the HAM activity window while Wc0+hT0
            # stream (first real matmul can't start before ~11us).
            for wi in range(12):
                wt = pswarm.tile([P, P], F16, name="warm", tag="warm")
                nc.tensor.transpose(wt[:], ident16, ident16)

            # --- hWT phase: all 16 groups (4 batches x 4 dout-pair groups).
            # Group (i, tp) depends only on W chunk tp/2 and hT_i, so batch
            # 0's groups chase the W-chunk DMAs during the ramp.
            hWT = [
                hwtpool.tile([P, NDC, TH], F16, name=f"hWT{i}", tag="hWT")
                for i in range(BPC)
            ]

            def emit_hwt_group(i, tp):
                """One tp-group (2 dout chunks) of hWT for batch i. 16 mm."""
                ps = pshw.tile([P, 512], F32, name="ps_hw", tag="pshw")
                for dt in range(2):
                    t = tp + dt
                    c, half = t // 2, t % 2
                    for j in range(NDC):
                        nc.tensor.matmul(
                            ps[:, dt * 256 : (dt + 1) * 256],
                            w16[:, c, j, half * P : (half + 1) * P],
                            ht_t[:, i, j, :],
                            start=(j == 0),
                            stop=(j == NDC - 1),
                        )
                nc.vector.tensor_copy(
                    hWT[i][:, tp : tp + 2, :].rearrange("p a b -> p (a b)"),
                    ps[:],
                )

            for i in range(BPC):
                for tp in range(0, NDC, 2):
                    emit_hwt_group(i, tp)

            # --- per-batch stream ---
            def make_batch(i):
                E = epool.tile([P, NQ, TB], F16, name=f"E{i}", tag="E")
                negmax = stats.tile([P, NQ, 1], F32, name="negmax", tag="negmax")
                S_sum = stats.tile([P, NQ, 1], F32, name="S_sum", tag="S")
                invS = stats.tile([P, NQ, 1], F32, name="invS", tag="invS")
                # attnT[p, r, c, q] = E[q, r, c*128+p]: one xbar per batch
                attnT = atpool.tile([P, NQ, NKC, P], F16, name=f"attnT{i}", tag="attnT")
                ctx16 = ctxpool.tile([P, NQ, D], F16, name=f"ctx{i}", tag="ctx")
                ps_scores = [None] * NQ

                def scores_mm(r, kh):
                    if ps_scores[r] is None:
                        ps_scores[r] = psbig.tile([P, TB], F32, name="ps_s", tag="psb")
                    ps_s = ps_scores[r]
                    for j in range(NDC):
                        nc.tensor.matmul(
                            ps_s[:, kh * 512 : (kh + 1) * 512],
                            hWT[i][:, j, r * P : (r + 1) * P],
                            bT[i][:, j, kh * 512 : (kh + 1) * 512],
                            start=(j == 0),
                            stop=(j == NDC - 1),
                        )

                def softmax_half(r):
                    # DVE rowmax -> ACT exp (rowsum via accum) -> DVE recip
                    ps_s = ps_scores[r]
                    nc.vector.tensor_reduce(
                        negmax[:, r, :],
                        ps_s[:],
                        axis=mybir.AxisListType.X,
                        op=mybir.AluOpType.max,
                        negate=True,
                    )
                    nc.scalar.activation(
                        E[:, r, :],
                        ps_s[:],
                        mybir.ActivationFunctionType.Exp,
                        bias=negmax[:, r, :],
                        accum_out=S_sum[:, r, :],
                    )
                    nc.vector.reciprocal(invS[:, r, :], S_sum[:, r, :])

                def xbar():
                    # whole-E transpose: in [128q, 2048(r,k)] -> out
                    # [128k, (r,c), 128q]
                    nc.scalar.dma_start(
                        attnT[:].rearrange("p r c q -> p (r c) q"),
                        E[:].rearrange("p r k -> p (r k)"),
                        transpose=True,
                    )

                def ctx_mm(r):
                    # separate [P,512] PSUM tiles per half: the half-0
                    # epilogue (mul reads PSUM) must not carry a
                    # tile-granular WAR against the half-1 matmuls
                    for dh in range(2):
                        ps_h = pshw.tile([P, 512], F32, name="ps_cs", tag="pshw")
                        for c in range(NKC):
                            nc.tensor.matmul(
                                ps_h[:],
                                attnT[:, r, c, :],
                                bN[i][:, c, dh * 512 : (dh + 1) * 512],
                                start=(c == 0),
                                stop=(c == NKC - 1),
                            )
                        sl = slice(dh * 512, (dh + 1) * 512)
                        nc.scalar.mul(ctx16[:, r, sl], ps_h[:], invS[:, r, :])
                    nc.scalar.dma_start(
                        out_ext[i, r * P : (r + 1) * P, :], ctx16[:, r, :]
                    )

                return scores_mm, softmax_half, xbar, ctx_mm

            # PE stream: s0 s1 ctx0 s2 ctx1 s3 ctx2 ctx3.  Batch i's softmax
            # + xbar latency hides behind batch i+1's scores matmuls.
            ops = [make_batch(i) for i in range(BPC)]

            def emit_scores(i):
                scores_mm, softmax_half, xbar, _ = ops[i]
                scores_mm(0, 0)
                scores_mm(0, 1)
                softmax_half(0)
                scores_mm(1, 0)
                scores_mm(1, 1)
                softmax_half(1)
                xbar()

            def emit_ctx(i):
                _, _, _, ctx_mm = ops[i]
                ctx_mm(0)
                ctx_mm(1)

            emit_scores(0)
            emit_scores(1)
            emit_ctx(0)
            emit_scores(2)
            emit_ctx(1)
            emit_scores(3)
            emit_ctx(2)
            emit_ctx(3)
    _split_excess_waits(nc)
    return nc


_NC_CACHE = None


def _get_nc():
    global _NC_CACHE
    if _NC_CACHE is None:
        _NC_CACHE = build_nc()
    return _NC_CACHE


def run(b, h, W_b, trace=False):
    """Shard, execute on 8 cores, gather. Returns (ctx, BassKernelResults)."""
    assert b.shape == (B, TB, D) and h.shape == (B, TH, D)
    # All on-chip compute is fp16; cast and pre-pack on the host so every
    # DMA moves >=4KB contiguous per partition and the PE never does layout.
    W16 = W_b[0].astype(np.float16)  # [D, D]
    # w[c, p, j, d] = W[j*128+p, c*256+d]
    wr = np.ascontiguousarray(
        W16.reshape(NDC, P, 4, 256).transpose(2, 1, 0, 3)
    )
    h16 = h.astype(np.float16)
    # hT[i, p, c, q] = h[i, q, c*128+p]
    hTr = np.ascontiguousarray(h16.reshape(B, TH, NDC, P).transpose(0, 3, 2, 1))
    b16 = b.astype(np.float16)
    # bT[i, p, c, k] = b[i, k, c*128+p]
    bTr = np.ascontiguousarray(b16.reshape(B, TB, NDC, P).transpose(0, 3, 2, 1))
    # bn[i, p, c, d] = b[i, c*128+p, d]
    bnr = np.ascontiguousarray(b16.reshape(B, NKC, P, D).transpose(0, 2, 1, 3))
    ident = np.eye(P, dtype=np.float16)
    in_maps = []
    for c in range(N_CORES):
        sl = slice(c * BPC, (c + 1) * BPC)
        in_maps.append(
            {
                "b": bnr[sl],
                "bT": bTr[sl],
                "hT": hTr[sl],
                "w": wr,
                "ident": ident,
            }
        )
    res = run_bass_kernel_spmd(
        _get_nc(), in_maps, core_ids=list(range(N_CORES)), trace=trace
    )
    out = np.concatenate([res.results[c]["out"] for c in range(N_CORES)], axis=0)
    return out.astype(np.float32), res


def kernel(b, h, W_b):
    out, _ = run(b, h, W_b, trace=False)
    return out


# revision 6
# speedup vs baseline: 1.0713x; 1.0713x over previous
"""Trainium2 Bass kernel for nn_Attention_3513283248742.

Bilinear attention: scores = h @ W @ b^T, attn = softmax(scores, -1),
ctx = attn @ b.  Shapes: b [32,1024,1024], h [32,256,1024], W_b [1,1024,1024].

Sharding: data-parallel over batch B=32 across 8 NeuronCores (4 batches per
core); W replicated.  No collectives.

v4 over v2 (110.7us baseline):
  * attnT transposes move from the PE to the DMA xbar (one
    dma_start(transpose=True) per batch on the ACT HWDGE queue), cutting
    ~6us of PE time; the PE runs only matmuls (plus a tiny HAM warmup).
  * All 16 hWT groups (4 batches) run FIRST: their DMA deps are just
    W (2MB) + hT (2MB), so the PE has ~27us of work queued by t=12us while
    the 16MB of bT/b streams far ahead of the scores/ctx consumers.
  * Every input is host-packed so each DMA writes 4-16KB contiguous rows
    per partition (v2's 512B-row descriptors ran the ramp at half rate).
  * No SBUF-reader gates and bufs=4 on the bT/b pools: the input queue is
    pure FIFO with zero instruction-level waits, so it never head-of-line
    blocks (v3's pool-reuse waits starved the late batches).
  * PE stream after the hWT phase: s0 s1 ctx0 s2 ctx1 s3 ctx2 ctx3 -- each
    batch's softmax+xbar latency hides behind the next batch's scores.

Per-core pipeline (per batch i):
  hWT  = W^T @ hT_i           lhsT = W chunks, rhs = hT (from host)
  S    = hWT^T @ bT_i         scores [q,k]
  softmax over k: DVE row max, ACT exp (+rowsum via accum), DVE recip
  attnT = DMA xbar transpose of E (fp16, SBUF->SBUF)
  ctx  = attnT^T @ b_i        rhs = b natural layout (from host)
  out  = ctx * invS           ACT epilogue, fp16, DMA'd from the ACT queue
"""

import numpy as np

import concourse.bass as bass
import concourse.mybir as mybir
import concourse.tile as tile
from concourse.bass_utils import run_bass_kernel_spmd
from concourse.vector_clock import ScopedClock

F32 = mybir.dt.float32
F16 = mybir.dt.float16

N_CORES = 8
B, TB, TH, D = 32, 1024, 256, 1024
BPC = B // N_CORES  # batches per core = 4
P = 128
NDC = D // P   # 8 chunks of the D axis
NKC = TB // P  # 8 chunks of the k axis
NQ = TH // P   # 2 chunks of the q axis

_PATCHED = False
CLEAR_SEMS_ON_EXIT = False


def _patch_tile_drain(max_waits_per_inst: int = 1):
    """This walrus build rejects >1 sem wait on the SP Drain instruction that
    TileContext emits on exit; split the waits across preceding sync nops."""
    global _PATCHED
    if _PATCHED:
        return
    _PATCHED = True

    def _drain_and_barrier(self, tick_clock, wait_clock):
        nc = self.nc
        drain_inst = nc.sync.drain()
        wait_clock.add_sem_waits(
            drain_inst.ins, ScopedClock({None: tick_clock.global_clock})
        )
        si = drain_inst.ins.sync_info
        if si is not None and si.on_wait and len(si.on_wait) > max_waits_per_inst:
            waits = list(si.on_wait)
            bb = nc.cur_bb.bb
            assert bb.instructions[-1] is drain_inst.ins
            bb.instructions.pop()
            si.on_wait = waits[:max_waits_per_inst]
            rest = waits[max_waits_per_inst:]
            for i in range(0, len(rest), max_waits_per_inst):
                nop = nc.sync.nop(nofuse=True)
                chunk = rest[i : i + max_waits_per_inst]
                if nop.ins.sync_info is None:
                    nop.ins.sync_info = mybir.SyncInfo(on_wait=chunk, on_update=[])
                else:
                    nop.ins.sync_info.on_wait.extend(chunk)
            bb.instructions.append(drain_inst.ins)
        nc.all_engine_barrier()
        assert self.sems is not None
        popped = nc._tile_sem_poison_stack.pop()
        assert popped is self._sem_poison
        if CLEAR_SEMS_ON_EXIT:
            nc.clear_and_free_semaphores(list(self.sems.allocated().values()))
            nc.all_engine_barrier()
        else:
            nc._state.prepend_free_semaphores(
                [
                    s.num if hasattr(s, "num") else s
                    for s in self.sems.allocated().values()
                ]
            )

    tile.TileContext._drain_and_barrier = _drain_and_barrier


def _split_excess_waits(nc, max_waits: int = 1):
    """Walrus rejects instructions carrying more than `max_waits` sem waits.
    Hoist excess waits onto same-engine nops inserted just before."""
    for f in nc.m.functions:
        for bb in f.blocks:
            out = []
            for ins in list(bb.instructions):
                si = ins.sync_info
                if si is not None and si.on_wait and len(si.on_wait) > max_waits:
                    waits = list(si.on_wait)
                    si.on_wait = waits[:max_waits]
                    rest = waits[max_waits:]
                    for i in range(0, len(rest), max_waits):
                        nop = nc.engines[ins.engine].nop(nofuse=True)
                        cur_bb = nc.cur_bb.bb
                        assert cur_bb.instructions[-1] is nop.ins
                        cur_bb.instructions.pop()
                        nop.ins.sync_info = mybir.SyncInfo(
                            on_wait=rest[i : i + max_waits], on_update=[]
                        )
                        out.append(nop.ins)
                out.append(ins)
            bb.instructions[:] = out


def build_nc():
    _patch_tile_drain()
    nc = bass.Bass(trn_type="TRN2", target_bir_lowering=False, debug=False)
    # all inputs host-packed so each DMA is [P, ...] with >=4KB contiguous
    # per partition on BOTH sides (fast descriptors)
    b_ext = nc.declare_dram_parameter("b", [BPC, P, NKC, D], F16, isOutput=False)
    bt_ext = nc.declare_dram_parameter("bT", [BPC, P, NDC, TB], F16, isOutput=False)
    ht_ext = nc.declare_dram_parameter("hT", [BPC, P, NDC, TH], F16, isOutput=False)
    w_ext = nc.declare_dram_parameter("w", [4, P, NDC, 256], F16, isOutput=False)
    ident_ext = nc.declare_dram_parameter("ident", [P, P], F16, isOutput=False)
    out_ext = nc.declare_dram_parameter("out", [BPC, TH, D], F16, isOutput=True)

    with tile.TileContext(nc) as tc:
        with (
            tc.tile_pool(name="consts", bufs=1) as consts,
            tc.tile_pool(name="bpool", bufs=4) as bpool,
            tc.tile_pool(name="btpool", bufs=4) as btpool,
            tc.tile_pool(name="hwtpool", bufs=4) as hwtpool,
            tc.tile_pool(name="epool", bufs=2) as epool,
            tc.tile_pool(name="atpool", bufs=2) as atpool,
            tc.tile_pool(name="ctxpool", bufs=2) as ctxpool,
            tc.tile_pool(name="stats", bufs=2) as stats,
            tc.tile_pool(name="psbig", bufs=2, space="PSUM") as psbig,
            tc.tile_pool(name="pshw", bufs=2, space="PSUM") as pshw,
            tc.tile_pool(name="pswarm", bufs=2, space="PSUM") as pswarm,
        ):
            # ALL input loads ride the SWDGE (Pool/gpsimd) ring in strict
            # priority order.  SWDGE uses its own DMASW completion-sem lanes,
            # so input issues can never chain onto the compute-gated xbar/out
            # DMAs on the ACT HWDGE ring (v4's bT3/b3 issue stalled ~20us on
            # exactly that lane reuse).  17 input DMAs cycle 8 DMASW lanes;
            # every reuse waits on an *early* input, which is free.
            ident_t = consts.tile([P, P], F16)
            nc.gpsimd.dma_start(ident_t[:], ident_ext.ap())
            ident16 = ident_t[:]

            # W in SBUF chunk-major: [din(part), chunk, j, dout256]
            w16 = consts.tile([P, 4, NDC, 256], F16)
            # all four hT batches share one tile, one DMA per batch
            ht_t = consts.tile([P, BPC, NDC, TH], F16)

            bT = [None] * BPC
            bN = [None] * BPC
            # ramp-critical chase order: Wc0, hT0, Wc1, hT1, Wc2, Wc3, ...
            ramp = [(0, "w"), (0, "h"), (1, "w"), (1, "h"), (2, "w"),
                    (3, "w"), (2, "h"), (3, "h")]
            for i, kind in ramp:
                if kind == "w":
                    nc.gpsimd.dma_start(w16[:, i], w_ext[i])
                else:
                    nc.gpsimd.dma_start(ht_t[:, i], ht_ext[i])
            for i in range(BPC):
                bT[i] = btpool.tile([P, NDC, TB], F16, name=f"bT{i}", tag="bT")
                nc.gpsimd.dma_start(bT[i][:], bt_ext[i])
                bN[i] = bpool.tile([P, NKC, D], F16, name=f"b{i}", tag="b")
                nc.gpsimd.dma_start(bN[i][:], b_ext[i])

            # --- PE warmup: trip the HAM activity window while Wc0+hT0
            # stream (first real matmul can't start before ~11us).
            for wi in range(12):
                wt = pswarm.tile([P, P], F16, name="warm", tag="warm")
                nc.tensor.transpose(wt[:], ident16, ident16)

            # --- hWT phase: all 16 groups (4 batches x 4 dout-pair groups).
            # Group (i, tp) depends only on W chunk tp/2 and hT_i, so batch
            # 0's groups chase the W-chunk DMAs during the ramp.
            hWT = [
                hwtpool.tile([P, NDC, TH], F16, name=f"hWT{i}", tag="hWT")
                for i in range(BPC)
            ]

            def emit_hwt_group(i, tp):
                """One tp-group (2 dout chunks) of hWT for batch i. 16 mm."""
                ps = pshw.tile([P, 512], F32, name="ps_hw", tag="pshw")
                for dt in range(2):
                    t = tp + dt
                    c, half = t // 2, t % 2
                    for j in range(NDC):
                        nc.tensor.matmul(
                            ps[:, dt * 256 : (dt + 1) * 256],
                            w16[:, c, j, half * P : (half + 1) * P],
                            ht_t[:, i, j, :],
                            start=(j == 0),
                            stop=(j == NDC - 1),
                        )
                nc.vector.tensor_copy(
                    hWT[i][:, tp : tp + 2, :].rearrange("p a b -> p (a b)"),
                    ps[:],
                )

            for i in range(BPC):
                for tp in range(0, NDC, 2):
                    emit_hwt_group(i, tp)

            # --- per-batch stream ---
            def make_batch(i):
                E = epool.tile([P, NQ, TB], F16, name=f"E{i}", tag="E")
                negmax = stats.tile([P, NQ, 1], F32, name="negmax", tag="negmax")
                S_sum = stats.tile([P, NQ, 1], F32, name="S_sum", tag="S")
                invS = stats.tile([P, NQ, 1], F32, name="invS", tag="invS")
                # attnT[p, r, c, q] = E[q, r, c*128+p]: one xbar per batch
                attnT = atpool.tile([P, NQ, NKC, P], F16, name=f"attnT{i}", tag="attnT")
                ctx16 = ctxpool.tile([P, NQ, D], F16, name=f"ctx{i}", tag="ctx")
                ps_scores = [None] * NQ

                def scores_mm(r, kh):
                    if ps_scores[r] is None:
                        ps_scores[r] = psbig.tile([P, TB], F32, name="ps_s", tag="psb")
                    ps_s = ps_scores[r]
                    for j in range(NDC):
                        nc.tensor.matmul(
                            ps_s[:, kh * 512 : (kh + 1) * 512],
                            hWT[i][:, j, r * P : (r + 1) * P],
                            bT[i][:, j, kh * 512 : (kh + 1) * 512],
                            start=(j == 0),
                            stop=(j == NDC - 1),
                        )

                def softmax_half(r):
                    # DVE rowmax -> ACT exp (rowsum via accum) -> DVE recip
                    ps_s = ps_scores[r]
                    nc.vector.tensor_reduce(
                        negmax[:, r, :],
                        ps_s[:],
                        axis=mybir.AxisListType.X,
                        op=mybir.AluOpType.max,
                        negate=True,
                    )
                    nc.scalar.activation(
                        E[:, r, :],
                        ps_s[:],
                        mybir.ActivationFunctionType.Exp,
                        bias=negmax[:, r, :],
                        accum_out=S_sum[:, r, :],
                    )
                    nc.vector.reciprocal(invS[:, r, :], S_sum[:, r, :])

                def xbar():
                    # whole-E transpose: in [128q, 2048(r,k)] -> out
                    # [128k, (r,c), 128q]
                    nc.scalar.dma_start(
                        attnT[:].rearrange("p r c q -> p (r c) q"),
                        E[:].rearrange("p r k -> p (r k)"),
                        transpose=True,
                    )

                def ctx_mm(r):
                    # separate [P,512] PSUM tiles per half: the half-0
                    # epilogue (mul reads PSUM) must not carry a
                    # tile-granular WAR against the half-1 matmuls
                    for dh in range(2):
                        ps_h = pshw.tile([P, 512], F32, name="ps_cs", tag="pshw")
                        for c in range(NKC):
                            nc.tensor.matmul(
                                ps_h[:],
                                attnT[:, r, c, :],
                                bN[i][:, c, dh * 512 : (dh + 1) * 512],
                                start=(c == 0),
                                stop=(c == NKC - 1),
                            )
                        sl = slice(dh * 512, (dh + 1) * 512)
                        nc.scalar.mul(ctx16[:, r, sl], ps_h[:], invS[:, r, :])
                    nc.scalar.dma_start(
                        out_ext[i, r * P : (r + 1) * P, :], ctx16[:, r, :]
                    )

                return scores_mm, softmax_half, xbar, ctx_mm

            # PE stream: s0 s1 ctx0 s2 ctx1 s3 ctx2 ctx3.  Batch i's softmax
            # + xbar latency hides behind batch i+1's scores matmuls.
            ops = [make_batch(i) for i in range(BPC)]

            def emit_scores(i):
                scores_mm, softmax_half, xbar, _ = ops[i]
                scores_mm(0, 0)
                scores_mm(0, 1)
                softmax_half(0)
                scores_mm(1, 0)
                scores_mm(1, 1)
                softmax_half(1)
                xbar()

            def emit_ctx(i):
                _, _, _, ctx_mm = ops[i]
                ctx_mm(0)
                ctx_mm(1)

            emit_scores(0)
            emit_scores(1)
            emit_ctx(0)
            emit_scores(2)
            emit_ctx(1)
            emit_scores(3)
            emit_ctx(2)
            emit_ctx(3)
    _split_excess_waits(nc)
    return nc


_NC_CACHE = None


def _get_nc():
    global _NC_CACHE
    if _NC_CACHE is None:
        _NC_CACHE = build_nc()
    return _NC_CACHE


def run(b, h, W_b, trace=False):
    """Shard, execute on 8 cores, gather. Returns (ctx, BassKernelResults)."""
    assert b.shape == (B, TB, D) and h.shape == (B, TH, D)
    # All on-chip compute is fp16; cast and pre-pack on the host so every
    # DMA moves >=4KB contiguous per partition and the PE never does layout.
    W16 = W_b[0].astype(np.float16)  # [D, D]
    # w[c, p, j, d] = W[j*128+p, c*256+d]
    wr = np.ascontiguousarray(
        W16.reshape(NDC, P, 4, 256).transpose(2, 1, 0, 3)
    )
    h16 = h.astype(np.float16)
    # hT[i, p, c, q] = h[i, q, c*128+p]
    hTr = np.ascontiguousarray(h16.reshape(B, TH, NDC, P).transpose(0, 3, 2, 1))
    b16 = b.astype(np.float16)
    # bT[i, p, c, k] = b[i, k, c*128+p]
    bTr = np.ascontiguousarray(b16.reshape(B, TB, NDC, P).transpose(0, 3, 2, 1))
    # bn[i, p, c, d] = b[i, c*128+p, d]
    bnr = np.ascontiguousarray(b16.reshape(B, NKC, P, D).transpose(0, 2, 1, 3))
    ident = np.eye(P, dtype=np.float16)
    in_maps = []
    for c in range(N_CORES):
        sl = slice(c * BPC, (c + 1) * BPC)
        in_maps.append(
            {
                "b": bnr[sl],
                "bT": bTr[sl],
                "hT": hTr[sl],
                "w": wr,
                "ident": ident,
            }
        )
    res = run_bass_kernel_spmd(
        _get_nc(), in_maps, core_ids=list(range(N_CORES)), trace=trace
    )
    out = np.concatenate([res.results[c]["out"] for c in range(N_CORES)], axis=0)
    return out.astype(np.float32), res


def kernel(b, h, W_b):
    out, _ = run(b, h, W_b, trace=False)
    return out
